# revision 7
# baseline (speedup 1.0000x reference)
"""ChemGeomFeatEncoder TRN2 kernel, v7.

Strategy: shard edges by OWNER VERTEX across 8 cores (host argsort of
nbr_vids).  Each core owns a contiguous V/8 vertex range and processes the
(sorted, padded) edges pointing into it.  One-hot scatter masks are
precomputed on host and streamed from HBM as bf16.

v7 vs v3-v6:
  * tanh moved from a custom DVE op to the Scalar engine's native Tanh
    table (silu_and_others holds both silu and tanh -> no table switch).
    The filter bias now rides a K=1 ones-row matmul into pf, same as pc.
    This halves the Vector engine load, which (with its per-op DRAIN)
    was pacing the whole pipeline at ~2.4us/supertile.
  * Scatter windows are evacuated in PAIRS: window 2w+1 overwrites the
    yet-unwritten half of the same PSUM bank (per-element has_written
    semantics), so one [128,128] DVE copy evacuates two windows.
  * Vertex phase (geom/feat MLPs) runs bf16.
  * Supertile stages are software-pipelined: at step i the PE runs
    mm1(i), mm2(i-1), scatter(i-2) so no PE instruction waits on a
    same-step cross-engine result.
"""
import numpy as np
import ml_dtypes

import concourse.bacc as bacc
import concourse.mybir as mybir
import concourse.tile as tile
from concourse.bass_utils import run_bass_kernel_spmd

dt = mybir.dt
AF = mybir.ActivationFunctionType
OP = mybir.AluOpType

EPS = 1e-5
NCORES = 8
P = 128          # partitions / tile edge dim
ST = 512         # supertile edge count (4 tiles)
CH = 8           # supertiles per chem/mask DMA
W = 64           # scatter window (vertices per PSUM accumulation region)
BF16 = ml_dtypes.bfloat16
DEBUG = False
TRACE = False
LAST_RESULT = None

_cache = {}

# ---------------------------------------------------------------------------
# Custom DVE op: fused softplus*gate.
# ---------------------------------------------------------------------------
_POLY = {}


def _register_dve_ops():
    from concourse.dve_spec import (
        Spec, Src0, Src1, One, C0, C1, C2, sq, lower, _has_src1 as has_src1)
    from concourse.dve_ops import DveOp, OPS, _SUB_OPCODE_FOR_NAME, CUSTOM_DVE_SPECS
    from concourse.dve_uop import DveOpSpec

    def reg(name, spec):
        if name in _SUB_OPCODE_FOR_NAME:
            return next(o for o in OPS if o.name == name)
        opcode = max(_SUB_OPCODE_FOR_NAME.values()) + 1
        shas = {}
        for ver in ("v3", "v4"):
            s = DveOpSpec(name=name, opcode=opcode, uops=lower(spec, ver=ver),
                          rd1_en=has_src1(spec))
            shas[ver] = s.sha(ver)
        op = DveOp(name, spec, subdim=False, uops_sha=shas)
        OPS.append(op)
        _SUB_OPCODE_FOR_NAME[name] = opcode
        CUSTOM_DVE_SPECS[name] = spec
        return op

    # GATE: out = (Src0 + e0 + u*(e1 + u*e2)) * (1 + Src1);  u = Src0^2
    #   Src0 = y_c (bias already accumulated in PSUM), Src1 = tanh tile.
    uc = sq(Src0)
    sp = Src0 + (C0 + uc * (C1 + uc * C2))
    gate_body = sp * (One + Src1)
    _POLY["GATE_SP"] = reg("GATE_SP", Spec(body=gate_body))


_register_dve_ops()


def _poly_fit(fn, R, degs, sig, n=80001):
    t = np.linspace(-R, R, n)
    w = np.exp(-0.5 * (t / sig) ** 2) + 0.02
    A = np.stack([t ** k for k in degs], axis=1)
    coef, *_ = np.linalg.lstsq(A * w[:, None], fn(t) * w, rcond=None)
    return [float(c) for c in coef]


# ln(2cosh(y)) on y in [-1.3,1.3] (actual |y|<=0.93), even deg-4
SP_COEF = _poly_fit(lambda y: np.log(2 * np.cosh(y)), 1.3, (0, 2, 4), sig=0.30)


def _fold(w, b, bn):
    """y = bn(x@w + b) -> x@w' + b' with eval-mode BN folded in."""
    g, be, m, v = bn[0], bn[1], bn[2], bn[3]
    a = g / np.sqrt(v + EPS)
    return (w * a[None, :]).astype(np.float32), ((b - m) * a + be).astype(np.float32)


def _host_prep(chem_feats, geom_feats, nbr_vids, weights):
    """Sort edges by vertex, build per-core padded streams + masks."""
    (w1, b1, bn1, w2, b2, bn2, wg1, bg1, bng1, wg2, bg2, bng2,
     wf1, bf1, bnf1, wf2, bf2, bnf2) = weights
    E, CHEM_IN = chem_feats.shape
    V, GEOM_IN = geom_feats.shape
    H = w1.shape[1]
    VC = V // NCORES
    NW = VC // W            # scatter windows per core

    w1f, b1f = _fold(w1, b1, bn1)
    w2f, b2f = _fold(w2, b2, bn2)
    wg1f, bg1f = _fold(wg1, bg1, bng1)
    wg2f, bg2f = _fold(wg2, bg2, bng2)
    wf1f, bf1f = _fold(wf1, bf1, bnf1)
    wf2f, bf2f = _fold(wf2, bf2, bnf2)
    # gate = sigma(f)*softplus(c) = 0.5*(1+tanh(f/2))*sp(c); fold the 0.5
    # into the h_chem rows of wf1.
    wf1f = wf1f.copy()
    wf1f[:H, :] *= 0.5
    # fold the /2 of both gate args into w2/b2 halves
    w2h = (0.5 * w2f).astype(BF16)
    b2h = 0.5 * b2f

    order = np.argsort(nbr_vids, kind="stable")
    svids = nbr_vids[order].astype(np.int64)

    # per-(core,window) edge counts; common tiles-per-window across cores
    win_bounds = np.searchsorted(svids, np.arange(NCORES * NW + 1) * W)
    win_counts = np.diff(win_bounds).reshape(NCORES, NW)
    T_w = np.maximum((win_counts + P - 1) // P, 1).max(axis=0)  # [NW]
    n_tiles = int(T_w.sum())
    # pad tile count to a 4*CH multiple so chem/mask DMAs batch evenly
    pad = (-n_tiles) % (4 * CH)
    T_w = T_w.copy()
    T_w[-1] += pad
    n_tiles += pad
    E_pad = n_tiles * P
    n_st = n_tiles // 4

    tile_off = np.zeros(NW + 1, dtype=np.int64)
    np.cumsum(T_w, out=tile_off[1:])

    chemT_pad = np.zeros((NCORES, CHEM_IN, E_pad), dtype=BF16)
    maskT = np.zeros((NCORES, P, n_tiles * W), dtype=BF16)
    chem_sorted = np.ascontiguousarray(chem_feats[order].T)  # [CHEM_IN, E] sorted
    for c in range(NCORES):
        cnts = win_counts[c]
        starts = win_bounds[c * NW:(c + 1) * NW]
        dst_col = np.concatenate(
            [tile_off[w] * P + np.arange(cnts[w]) for w in range(NW)])
        src_idx = np.concatenate(
            [starts[w] + np.arange(cnts[w]) for w in range(NW)])
        chemT_pad[c][:, dst_col] = chem_sorted[:, src_idx].astype(BF16)
        vrel = np.concatenate(
            [svids[starts[w]:starts[w] + cnts[w]] - (c * VC + w * W)
             for w in range(NW)])
        t_idx = dst_col // P
        e_row = dst_col % P
        maskT[c][e_row, t_idx * W + vrel] = 1.0

    geomT = np.ascontiguousarray(
        geom_feats.reshape(NCORES, VC, GEOM_IN).transpose(0, 2, 1)).astype(BF16)

    consts = dict(
        w1f=np.ascontiguousarray(w1f.astype(BF16)), b1f=b1f.reshape(H, 1),
        w2h_f=np.ascontiguousarray(w2h[:, :H]),
        w2h_c=np.ascontiguousarray(w2h[:, H:]),
        biasf_row=np.tile(b2h[:H], 4)[None, :].astype(BF16).copy(),
        biasc_row=np.tile(b2h[H:], 4)[None, :].astype(BF16).copy(),
        ones_row=np.ones((1, P), dtype=BF16),
        wg1f=np.ascontiguousarray(wg1f.astype(BF16)), bg1f=bg1f.reshape(-1, 1),
        wg2f=np.ascontiguousarray(wg2f.astype(BF16)), bg2f=bg2f.reshape(-1, 1),
        wf1f_a=np.ascontiguousarray(wf1f[:H, :].astype(BF16)),
        wf1f_b=np.ascontiguousarray(wf1f[H:, :].astype(BF16)),
        bf1f=bf1f.reshape(H, 1),
        wf2f=np.ascontiguousarray(wf2f.astype(BF16)), bf2f=bf2f.reshape(H, 1),
        ident_bf=np.eye(P, dtype=BF16),
    )
    dims = dict(E=E, V=V, H=H, CHEM_IN=CHEM_IN, GEOM_IN=GEOM_IN,
                VC=VC, NW=NW, n_tiles=n_tiles, n_st=n_st, E_pad=E_pad)
    per_core = dict(chemT=chemT_pad, maskT=maskT, geomT=geomT)
    return dims, tuple(int(t) for t in T_w), consts, per_core


def _build_nc(dims, T_w, trace_sim=False):
    H = dims["H"]
    CHEM_IN = dims["CHEM_IN"]
    GEOM_IN = dims["GEOM_IN"]
    VC = dims["VC"]
    NW = dims["NW"]
    n_tiles = dims["n_tiles"]
    n_st = dims["n_st"]
    E_pad = dims["E_pad"]
    GH = H // 2  # geom hidden = 64

    # tile index -> (window, first?, last?)
    tile_win = []
    for w in range(NW):
        for k in range(T_w[w]):
            tile_win.append((w, k == 0, k == T_w[w] - 1))
    assert len(tile_win) == n_tiles
    assert NW % 2 == 0

    sc0, sc1, sc2 = SP_COEF
    GATE_SP = _POLY["GATE_SP"]

    nc = bacc.Bacc("TRN2", target_bir_lowering=False)
    tc = tile.TileContext(nc, trace_sim=trace_sim)

    d_chemT = nc.dram_tensor("chemT", [CHEM_IN, E_pad], dt.bfloat16, kind="ExternalInput")
    d_maskT = nc.dram_tensor("maskT", [P, n_tiles * W], dt.bfloat16, kind="ExternalInput")
    d_geomT = nc.dram_tensor("geomT", [GEOM_IN, VC], dt.bfloat16, kind="ExternalInput")
    d_w1f = nc.dram_tensor("w1f", [CHEM_IN, H], dt.bfloat16, kind="ExternalInput")
    d_b1f = nc.dram_tensor("b1f", [H, 1], dt.float32, kind="ExternalInput")
    d_w2h_f = nc.dram_tensor("w2h_f", [H, H], dt.bfloat16, kind="ExternalInput")
    d_w2h_c = nc.dram_tensor("w2h_c", [H, H], dt.bfloat16, kind="ExternalInput")
    d_biasf = nc.dram_tensor("biasf_row", [1, 4 * H], dt.bfloat16, kind="ExternalInput")
    d_biasc = nc.dram_tensor("biasc_row", [1, 4 * H], dt.bfloat16, kind="ExternalInput")
    d_ones = nc.dram_tensor("ones_row", [1, P], dt.bfloat16, kind="ExternalInput")
    d_wg1f = nc.dram_tensor("wg1f", [GEOM_IN, GH], dt.bfloat16, kind="ExternalInput")
    d_bg1f = nc.dram_tensor("bg1f", [GH, 1], dt.float32, kind="ExternalInput")
    d_wg2f = nc.dram_tensor("wg2f", [GH, GH], dt.bfloat16, kind="ExternalInput")
    d_bg2f = nc.dram_tensor("bg2f", [GH, 1], dt.float32, kind="ExternalInput")
    d_wf1f_a = nc.dram_tensor("wf1f_a", [H, H], dt.bfloat16, kind="ExternalInput")
    d_wf1f_b = nc.dram_tensor("wf1f_b", [GH, H], dt.bfloat16, kind="ExternalInput")
    d_bf1f = nc.dram_tensor("bf1f", [H, 1], dt.float32, kind="ExternalInput")
    d_wf2f = nc.dram_tensor("wf2f", [H, H], dt.bfloat16, kind="ExternalInput")
    d_bf2f = nc.dram_tensor("bf2f", [H, 1], dt.float32, kind="ExternalInput")
    d_ident = nc.dram_tensor("ident_bf", [P, P], dt.bfloat16, kind="ExternalInput")
    d_out = nc.dram_tensor("out", [VC, H], dt.float32, kind="ExternalOutput")

    with tc:
        with (
            tc.tile_pool(name="const", bufs=1) as cpool,
            tc.tile_pool(name="persist", bufs=1) as ppool,
        ):
            t_w1f = cpool.tile([CHEM_IN, H], dt.bfloat16)
            nc.sync.dma_start(out=t_w1f[:], in_=d_w1f[:])
            t_b1f = cpool.tile([H, 1], dt.float32)
            nc.sync.dma_start(out=t_b1f[:], in_=d_b1f[:])
            t_w2h_f = cpool.tile([H, H], dt.bfloat16)
            nc.sync.dma_start(out=t_w2h_f[:], in_=d_w2h_f[:])
            t_w2h_c = cpool.tile([H, H], dt.bfloat16)
            nc.sync.dma_start(out=t_w2h_c[:], in_=d_w2h_c[:])
            t_biasf = cpool.tile([1, 4 * H], dt.bfloat16)
            nc.sync.dma_start(out=t_biasf[:], in_=d_biasf[:])
            t_biasc = cpool.tile([1, 4 * H], dt.bfloat16)
            nc.sync.dma_start(out=t_biasc[:], in_=d_biasc[:])
            t_ones = cpool.tile([1, P], dt.bfloat16)
            nc.sync.dma_start(out=t_ones[:], in_=d_ones[:])

            # persistent accumulation target: h_chem^T per vertex [H, VC]
            t_hcv = ppool.tile([H, VC], dt.bfloat16)

            with (
                tc.tile_pool(name="chem_in", bufs=3) as chpool,
                tc.tile_pool(name="mask_in", bufs=3) as mkpool,
                tc.tile_pool(name="h1", bufs=4) as h1pool,
                tc.tile_pool(name="tnh", bufs=3) as tpool,
                tc.tile_pool(name="g2", bufs=3) as gpool2,
                tc.tile_pool(name="psA", bufs=2, space="PSUM") as psA,
                tc.tile_pool(name="psF", bufs=2, space="PSUM") as psF,
                tc.tile_pool(name="psC", bufs=2, space="PSUM") as psC,
                tc.tile_pool(name="psS", bufs=2, space="PSUM") as psS,
            ):
                # Software pipeline: step i runs mm1(i) / mm2+gate(i-1) /
                # scatter(i-2) so the PE never waits on same-step results.
                seg_acc = {}
                ct = None
                mts = {}
                h1s = {}
                g2s = {}
                for i in range(n_st + 2):
                    st = i
                    if st < n_st:
                        if st % CH == 0:
                            ct = chpool.tile([CHEM_IN, CH * ST], dt.bfloat16,
                                             tag="ct")
                            nc.sync.dma_start(
                                out=ct[:], in_=d_chemT[:, st * ST:(st + CH) * ST])
                            mts[st // CH] = mkpool.tile(
                                [P, CH * 4 * W], dt.bfloat16, tag="mt",
                                name=f"mt_{st // CH}")
                            nc.sync.dma_start(
                                out=mts[st // CH][:],
                                in_=d_maskT[:, st * 4 * W:(st + CH) * 4 * W])
                        cs = (st % CH) * ST
                        p1 = psA.tile([P, ST], dt.float32, tag="p1")
                        nc.tensor.matmul(out=p1[:], lhsT=t_w1f[:],
                                         rhs=ct[:, cs:cs + ST],
                                         start=True, stop=True)
                        h1s[st] = h1pool.tile([P, ST], dt.bfloat16, tag="h1",
                                              name=f"h1_{st}")
                        nc.scalar.activation(h1s[st][:], p1[:], AF.Silu,
                                             bias=t_b1f[:, :1])
                    sm = i - 1    # mm2 + tanh + gate stage
                    if 0 <= sm < n_st:
                        h1 = h1s[sm]
                        # filter half: bias rides a K=1 ones-row matmul
                        pf = psF.tile([P, ST], dt.float32, tag="pf")
                        nc.tensor.matmul(out=pf[:], lhsT=t_ones[:],
                                         rhs=t_biasf[:], start=True, stop=False)
                        for k in range(4):
                            nc.tensor.matmul(out=pf[:, k * H:(k + 1) * H],
                                             lhsT=h1[:, k * P:(k + 1) * P],
                                             rhs=t_w2h_f[:], start=False,
                                             stop=(k == 3))
                        tnh = tpool.tile([P, ST], dt.bfloat16, tag="tnh")
                        nc.scalar.activation(tnh[:], pf[:], AF.Tanh)
                        # core half
                        pc = psC.tile([P, ST], dt.float32, tag="pc")
                        nc.tensor.matmul(out=pc[:], lhsT=t_ones[:],
                                         rhs=t_biasc[:], start=True, stop=False)
                        for k in range(4):
                            nc.tensor.matmul(out=pc[:, k * H:(k + 1) * H],
                                             lhsT=h1[:, k * P:(k + 1) * P],
                                             rhs=t_w2h_c[:], start=False,
                                             stop=(k == 3))
                        g2s[sm] = gpool2.tile([P, ST], dt.bfloat16, tag="g2",
                                              name=f"g2_{sm}")
                        nc.vector._custom_dve(GATE_SP, out=g2s[sm][:], in0=pc[:],
                                              in1=tnh[:], s0=sc0, s1=sc1,
                                              imm2=sc2)
                        del h1s[sm]
                    sc = i - 2    # scatter stage
                    if 0 <= sc < n_st:
                        g2 = g2s[sc]
                        mt = mts[sc // CH]
                        for k in range(4):
                            t_idx = sc * 4 + k
                            win, first, last = tile_win[t_idx]
                            pair = win // 2
                            lo = win % 2      # half of the paired PSUM bank
                            mc = (t_idx % (CH * 4)) * W
                            if first and lo == 0:
                                seg_acc[pair] = psS.tile(
                                    [P, 2 * W], dt.float32, tag="segacc",
                                    name=f"segacc_{pair}")
                            # window 2p fills cols [0,W) with start=True on its
                            # first tile (clears the bank); window 2p+1 fills
                            # cols [W,2W) with start=False -- per-element
                            # has_written makes the first write an overwrite.
                            nc.tensor.matmul(
                                out=seg_acc[pair][:, lo * W:(lo + 1) * W],
                                lhsT=g2[:, k * P:(k + 1) * P],
                                rhs=mt[:, mc:mc + W],
                                start=(first and lo == 0),
                                stop=(last and lo == 1),
                                skip_group_check=True)
                            if last and lo == 1:
                                nc.vector.tensor_copy(
                                    out=t_hcv[:, (win - 1) * W:(win + 1) * W],
                                    in_=seg_acc[pair][:])
                                del seg_acc[pair]
                        del g2s[sc]
                        if sc % CH == CH - 1:
                            del mts[sc // CH]

            # ---------------- vertex phase (bf16) ----------------
            with (
                tc.tile_pool(name="geom_in", bufs=2) as gpool,
                tc.tile_pool(name="vtmp", bufs=3) as vtpool,
                tc.tile_pool(name="vout", bufs=3) as vopool,
                tc.tile_pool(name="psV", bufs=1, space="PSUM") as psV,
                tc.tile_pool(name="vconst", bufs=1) as vcpool,
            ):
                t_wg1f = vcpool.tile([GEOM_IN, GH], dt.bfloat16)
                nc.sync.dma_start(out=t_wg1f[:], in_=d_wg1f[:])
                t_bg1f = vcpool.tile([GH, 1], dt.float32)
                nc.sync.dma_start(out=t_bg1f[:], in_=d_bg1f[:])
                t_wg2f = vcpool.tile([GH, GH], dt.bfloat16)
                nc.sync.dma_start(out=t_wg2f[:], in_=d_wg2f[:])
                t_bg2f = vcpool.tile([GH, 1], dt.float32)
                nc.sync.dma_start(out=t_bg2f[:], in_=d_bg2f[:])
                t_wf1f_a = vcpool.tile([H, H], dt.bfloat16)
                nc.sync.dma_start(out=t_wf1f_a[:], in_=d_wf1f_a[:])
                t_wf1f_b = vcpool.tile([GH, H], dt.bfloat16)
                nc.sync.dma_start(out=t_wf1f_b[:], in_=d_wf1f_b[:])
                t_bf1f = vcpool.tile([H, 1], dt.float32)
                nc.sync.dma_start(out=t_bf1f[:], in_=d_bf1f[:])
                t_wf2f = vcpool.tile([H, H], dt.bfloat16)
                nc.sync.dma_start(out=t_wf2f[:], in_=d_wf2f[:])
                t_bf2f = vcpool.tile([H, 1], dt.float32)
                nc.sync.dma_start(out=t_bf2f[:], in_=d_bf2f[:])
                t_ident = vcpool.tile([P, P], dt.bfloat16)
                nc.sync.dma_start(out=t_ident[:], in_=d_ident[:])

                for base in range(0, VC, ST):
                    Wc = min(ST, VC - base)
                    sl = slice(base, base + Wc)
                    gt = gpool.tile([GEOM_IN, Wc], dt.bfloat16, tag="gt")
                    nc.sync.dma_start(out=gt[:], in_=d_geomT[:, sl])
                    pg1 = psV.tile([GH, Wc], dt.float32, tag="pg1")
                    nc.tensor.matmul(out=pg1[:], lhsT=t_wg1f[:], rhs=gt[:],
                                     start=True, stop=True)
                    g1s = vtpool.tile([GH, Wc], dt.bfloat16, tag="g1s")
                    nc.scalar.activation(g1s[:], pg1[:], AF.Silu, bias=t_bg1f[:, :1])
                    pg2 = psV.tile([GH, Wc], dt.float32, tag="pg2")
                    nc.tensor.matmul(out=pg2[:], lhsT=t_wg2f[:], rhs=g1s[:],
                                     start=True, stop=True)
                    hg = vtpool.tile([GH, Wc], dt.bfloat16, tag="hg")
                    nc.scalar.activation(hg[:], pg2[:], AF.Identity, bias=t_bg2f[:, :1])
                    # feat mlp
                    pf1 = psV.tile([H, Wc], dt.float32, tag="pf1", bufs=2)
                    nc.tensor.matmul(out=pf1[:], lhsT=t_wf1f_a[:],
                                     rhs=t_hcv[:, sl],
                                     start=True, stop=False)
                    nc.tensor.matmul(out=pf1[:], lhsT=t_wf1f_b[:], rhs=hg[:],
                                     start=False, stop=True)
                    x1 = vtpool.tile([H, Wc], dt.bfloat16, tag="x1")
                    nc.scalar.activation(x1[:], pf1[:], AF.Silu, bias=t_bf1f[:, :1])
                    pf2 = psV.tile([H, Wc], dt.float32, tag="pf2", bufs=2)
                    nc.tensor.matmul(out=pf2[:], lhsT=t_wf2f[:], rhs=x1[:],
                                     start=True, stop=True)
                    outT = vtpool.tile([H, Wc], dt.bfloat16, tag="outT")
                    nc.scalar.activation(outT[:], pf2[:], AF.Identity,
                                         bias=t_bf2f[:, :1])
                    for k in range(Wc // P):
                        trv = psV.tile([P, P], dt.bfloat16, tag="trv", bufs=2)
                        nc.tensor.transpose(
                            out=trv[:], in_=outT[:, k * P:(k + 1) * P],
                            identity=t_ident[:])
                        ov = vopool.tile([P, H], dt.float32, tag="ov")
                        nc.vector.tensor_copy(out=ov[:], in_=trv[:])
                        nc.sync.dma_start(
                            out=d_out[base + k * P: base + (k + 1) * P, :],
                            in_=ov[:])

    nc.compile()
    if trace_sim:
        ents = [e for e in tc._perfetto_entries if e[2] is not None]
        if ents:
            t0 = min(e[1] for e in ents)
            t1 = max(e[2] for e in ents)
            print(f"[sim] estimated makespan: {(t1 - t0) / 1000:.1f} us")
            nc._sim_makespan_ns = t1 - t0
    return nc


def kernel(chem_feats, geom_feats, nbr_vids,
           w1, b1, bn1, w2, b2, bn2,
           wg1, bg1, bng1, wg2, bg2, bng2,
           wf1, bf1, bnf1, wf2, bf2, bnf2):
    chem_feats = np.asarray(chem_feats, dtype=np.float32)
    geom_feats = np.asarray(geom_feats, dtype=np.float32)
    nbr_vids = np.asarray(nbr_vids)
    weights = tuple(np.asarray(w, dtype=np.float32) for w in (
        w1, b1, bn1, w2, b2, bn2, wg1, bg1, bng1, wg2, bg2, bng2,
        wf1, bf1, bnf1, wf2, bf2, bnf2))

    dims, T_w, consts, per_core = _host_prep(
        chem_feats, geom_feats, nbr_vids, weights)

    key = (dims["E_pad"], T_w)
    if key not in _cache:
        _cache[key] = _build_nc(dims, T_w)
    nc = _cache[key]

    base = dict(consts)
    in_maps = []
    for c in range(NCORES):
        m = dict(base)
        m["chemT"] = per_core["chemT"][c]
        m["maskT"] = per_core["maskT"][c]
        m["geomT"] = per_core["geomT"][c]
        in_maps.append(m)

    global LAST_RESULT
    if TRACE:
        res = run_bass_kernel_spmd(nc, in_maps, core_ids=list(range(NCORES)),
                                   trace=True, tmpdir="/tmp/bass_trace")
    else:
        res = run_bass_kernel_spmd(nc, in_maps, core_ids=list(range(NCORES)))
    LAST_RESULT = res
    out = np.concatenate([r["out"] for r in res.results], axis=0)
    return out.astype(np.float32)


# revision 8
# speedup vs baseline: 1.0023x; 1.0023x over previous
"""ChemGeomFeatEncoder TRN2 kernel, v7.

Strategy: shard edges by OWNER VERTEX across 8 cores (host argsort of
nbr_vids).  Each core owns a contiguous V/8 vertex range and processes the
(sorted, padded) edges pointing into it.  One-hot scatter masks are
precomputed on host and streamed from HBM as bf16.

v7 vs v3-v6:
  * tanh moved from a custom DVE op to the Scalar engine's native Tanh
    table (silu_and_others holds both silu and tanh -> no table switch).
    The filter bias now rides a K=1 ones-row matmul into pf, same as pc.
    This halves the Vector engine load, which (with its per-op DRAIN)
    was pacing the whole pipeline at ~2.4us/supertile.
  * Scatter windows are evacuated in PAIRS: window 2w+1 overwrites the
    yet-unwritten half of the same PSUM bank (per-element has_written
    semantics), so one [128,128] DVE copy evacuates two windows.
  * Vertex phase (geom/feat MLPs) runs bf16.
  * Supertile stages are software-pipelined: at step i the PE runs
    mm1(i), mm2(i-1), scatter(i-2) so no PE instruction waits on a
    same-step cross-engine result.
"""
import numpy as np
import ml_dtypes

import concourse.bacc as bacc
import concourse.mybir as mybir
import concourse.tile as tile
from concourse.bass_utils import run_bass_kernel_spmd

dt = mybir.dt
AF = mybir.ActivationFunctionType
OP = mybir.AluOpType

EPS = 1e-5
NCORES = 8
P = 128          # partitions / tile edge dim
ST = 512         # supertile edge count (4 tiles)
CH = 8           # supertiles per chem/mask DMA
W = 64           # scatter window (vertices per PSUM accumulation region)
BF16 = ml_dtypes.bfloat16
DEBUG = False
TRACE = False
LAST_RESULT = None

_cache = {}

# ---------------------------------------------------------------------------
# Custom DVE op: fused softplus*gate.
# ---------------------------------------------------------------------------
_POLY = {}


def _register_dve_ops():
    from concourse.dve_spec import (
        Spec, Src0, Src1, One, C0, C1, C2, sq, lower, _has_src1 as has_src1)
    from concourse.dve_ops import DveOp, OPS, _SUB_OPCODE_FOR_NAME, CUSTOM_DVE_SPECS
    from concourse.dve_uop import DveOpSpec

    def reg(name, spec):
        if name in _SUB_OPCODE_FOR_NAME:
            return next(o for o in OPS if o.name == name)
        opcode = max(_SUB_OPCODE_FOR_NAME.values()) + 1
        shas = {}
        for ver in ("v3", "v4"):
            s = DveOpSpec(name=name, opcode=opcode, uops=lower(spec, ver=ver),
                          rd1_en=has_src1(spec))
            shas[ver] = s.sha(ver)
        op = DveOp(name, spec, subdim=False, uops_sha=shas)
        OPS.append(op)
        _SUB_OPCODE_FOR_NAME[name] = opcode
        CUSTOM_DVE_SPECS[name] = spec
        return op

    # GATE: out = (Src0 + e0 + u*(e1 + u*e2)) * (1 + Src1);  u = Src0^2
    #   Src0 = y_c (bias already accumulated in PSUM), Src1 = tanh tile.
    uc = sq(Src0)
    sp = Src0 + (C0 + uc * (C1 + uc * C2))
    gate_body = sp * (One + Src1)
    _POLY["GATE_SP"] = reg("GATE_SP", Spec(body=gate_body))


_register_dve_ops()


def _poly_fit(fn, R, degs, sig, n=80001):
    t = np.linspace(-R, R, n)
    w = np.exp(-0.5 * (t / sig) ** 2) + 0.02
    A = np.stack([t ** k for k in degs], axis=1)
    coef, *_ = np.linalg.lstsq(A * w[:, None], fn(t) * w, rcond=None)
    return [float(c) for c in coef]


# ln(2cosh(y)) on y in [-1.3,1.3] (actual |y|<=0.93), even deg-4
SP_COEF = _poly_fit(lambda y: np.log(2 * np.cosh(y)), 1.3, (0, 2, 4), sig=0.30)


def _fold(w, b, bn):
    """y = bn(x@w + b) -> x@w' + b' with eval-mode BN folded in."""
    g, be, m, v = bn[0], bn[1], bn[2], bn[3]
    a = g / np.sqrt(v + EPS)
    return (w * a[None, :]).astype(np.float32), ((b - m) * a + be).astype(np.float32)


def _host_prep(chem_feats, geom_feats, nbr_vids, weights):
    """Sort edges by vertex, build per-core padded streams + masks."""
    (w1, b1, bn1, w2, b2, bn2, wg1, bg1, bng1, wg2, bg2, bng2,
     wf1, bf1, bnf1, wf2, bf2, bnf2) = weights
    E, CHEM_IN = chem_feats.shape
    V, GEOM_IN = geom_feats.shape
    H = w1.shape[1]
    VC = V // NCORES
    NW = VC // W            # scatter windows per core

    w1f, b1f = _fold(w1, b1, bn1)
    w2f, b2f = _fold(w2, b2, bn2)
    wg1f, bg1f = _fold(wg1, bg1, bng1)
    wg2f, bg2f = _fold(wg2, bg2, bng2)
    wf1f, bf1f = _fold(wf1, bf1, bnf1)
    wf2f, bf2f = _fold(wf2, bf2, bnf2)
    # gate = sigma(f)*softplus(c) = 0.5*(1+tanh(f/2))*sp(c); fold the 0.5
    # into the h_chem rows of wf1.
    wf1f = wf1f.copy()
    wf1f[:H, :] *= 0.5
    # fold the /2 of both gate args into w2/b2 halves
    w2h = (0.5 * w2f).astype(BF16)
    b2h = 0.5 * b2f

    order = np.argsort(nbr_vids, kind="stable")
    svids = nbr_vids[order].astype(np.int64)

    # per-(core,window) edge counts; common tiles-per-window across cores
    win_bounds = np.searchsorted(svids, np.arange(NCORES * NW + 1) * W)
    win_counts = np.diff(win_bounds).reshape(NCORES, NW)
    T_w = np.maximum((win_counts + P - 1) // P, 1).max(axis=0)  # [NW]
    n_tiles = int(T_w.sum())
    # pad tile count to a 4*CH multiple so chem/mask DMAs batch evenly
    pad = (-n_tiles) % (4 * CH)
    T_w = T_w.copy()
    T_w[-1] += pad
    n_tiles += pad
    E_pad = n_tiles * P
    n_st = n_tiles // 4

    tile_off = np.zeros(NW + 1, dtype=np.int64)
    np.cumsum(T_w, out=tile_off[1:])

    chemT_pad = np.zeros((NCORES, CHEM_IN, E_pad), dtype=BF16)
    maskT = np.zeros((NCORES, P, n_tiles * W), dtype=BF16)
    chem_sorted = np.ascontiguousarray(chem_feats[order].T)  # [CHEM_IN, E] sorted
    for c in range(NCORES):
        cnts = win_counts[c]
        starts = win_bounds[c * NW:(c + 1) * NW]
        dst_col = np.concatenate(
            [tile_off[w] * P + np.arange(cnts[w]) for w in range(NW)])
        src_idx = np.concatenate(
            [starts[w] + np.arange(cnts[w]) for w in range(NW)])
        chemT_pad[c][:, dst_col] = chem_sorted[:, src_idx].astype(BF16)
        vrel = np.concatenate(
            [svids[starts[w]:starts[w] + cnts[w]] - (c * VC + w * W)
             for w in range(NW)])
        t_idx = dst_col // P
        e_row = dst_col % P
        maskT[c][e_row, t_idx * W + vrel] = 1.0

    geomT = np.ascontiguousarray(
        geom_feats.reshape(NCORES, VC, GEOM_IN).transpose(0, 2, 1)).astype(BF16)

    consts = dict(
        w1f=np.ascontiguousarray(w1f.astype(BF16)), b1f=b1f.reshape(H, 1),
        w2h_f=np.ascontiguousarray(w2h[:, :H]),
        w2h_c=np.ascontiguousarray(w2h[:, H:]),
        biasf_row=np.tile(b2h[:H], 4)[None, :].astype(BF16).copy(),
        biasc_row=np.tile(b2h[H:], 4)[None, :].astype(BF16).copy(),
        ones_row=np.ones((1, P), dtype=BF16),
        wg1f=np.ascontiguousarray(wg1f.astype(BF16)), bg1f=bg1f.reshape(-1, 1),
        wg2f=np.ascontiguousarray(wg2f.astype(BF16)), bg2f=bg2f.reshape(-1, 1),
        wf1f_a=np.ascontiguousarray(wf1f[:H, :].astype(BF16)),
        wf1f_b=np.ascontiguousarray(wf1f[H:, :].astype(BF16)),
        bf1f=bf1f.reshape(H, 1),
        wf2f=np.ascontiguousarray(wf2f.astype(BF16)), bf2f=bf2f.reshape(H, 1),
        ident_bf=np.eye(P, dtype=BF16),
    )
    dims = dict(E=E, V=V, H=H, CHEM_IN=CHEM_IN, GEOM_IN=GEOM_IN,
                VC=VC, NW=NW, n_tiles=n_tiles, n_st=n_st, E_pad=E_pad)
    per_core = dict(chemT=chemT_pad, maskT=maskT, geomT=geomT)
    return dims, tuple(int(t) for t in T_w), consts, per_core


def _build_nc(dims, T_w, trace_sim=False):
    H = dims["H"]
    CHEM_IN = dims["CHEM_IN"]
    GEOM_IN = dims["GEOM_IN"]
    VC = dims["VC"]
    NW = dims["NW"]
    n_tiles = dims["n_tiles"]
    n_st = dims["n_st"]
    E_pad = dims["E_pad"]
    GH = H // 2  # geom hidden = 64

    # tile index -> (window, first?, last?)
    tile_win = []
    for w in range(NW):
        for k in range(T_w[w]):
            tile_win.append((w, k == 0, k == T_w[w] - 1))
    assert len(tile_win) == n_tiles
    assert NW % 2 == 0

    sc0, sc1, sc2 = SP_COEF
    GATE_SP = _POLY["GATE_SP"]

    nc = bacc.Bacc("TRN2", target_bir_lowering=False)
    tc = tile.TileContext(nc, trace_sim=trace_sim)

    d_chemT = nc.dram_tensor("chemT", [CHEM_IN, E_pad], dt.bfloat16, kind="ExternalInput")
    d_maskT = nc.dram_tensor("maskT", [P, n_tiles * W], dt.bfloat16, kind="ExternalInput")
    d_geomT = nc.dram_tensor("geomT", [GEOM_IN, VC], dt.bfloat16, kind="ExternalInput")
    d_w1f = nc.dram_tensor("w1f", [CHEM_IN, H], dt.bfloat16, kind="ExternalInput")
    d_b1f = nc.dram_tensor("b1f", [H, 1], dt.float32, kind="ExternalInput")
    d_w2h_f = nc.dram_tensor("w2h_f", [H, H], dt.bfloat16, kind="ExternalInput")
    d_w2h_c = nc.dram_tensor("w2h_c", [H, H], dt.bfloat16, kind="ExternalInput")
    d_biasf = nc.dram_tensor("biasf_row", [1, 4 * H], dt.bfloat16, kind="ExternalInput")
    d_biasc = nc.dram_tensor("biasc_row", [1, 4 * H], dt.bfloat16, kind="ExternalInput")
    d_ones = nc.dram_tensor("ones_row", [1, P], dt.bfloat16, kind="ExternalInput")
    d_wg1f = nc.dram_tensor("wg1f", [GEOM_IN, GH], dt.bfloat16, kind="ExternalInput")
    d_bg1f = nc.dram_tensor("bg1f", [GH, 1], dt.float32, kind="ExternalInput")
    d_wg2f = nc.dram_tensor("wg2f", [GH, GH], dt.bfloat16, kind="ExternalInput")
    d_bg2f = nc.dram_tensor("bg2f", [GH, 1], dt.float32, kind="ExternalInput")
    d_wf1f_a = nc.dram_tensor("wf1f_a", [H, H], dt.bfloat16, kind="ExternalInput")
    d_wf1f_b = nc.dram_tensor("wf1f_b", [GH, H], dt.bfloat16, kind="ExternalInput")
    d_bf1f = nc.dram_tensor("bf1f", [H, 1], dt.float32, kind="ExternalInput")
    d_wf2f = nc.dram_tensor("wf2f", [H, H], dt.bfloat16, kind="ExternalInput")
    d_bf2f = nc.dram_tensor("bf2f", [H, 1], dt.float32, kind="ExternalInput")
    d_ident = nc.dram_tensor("ident_bf", [P, P], dt.bfloat16, kind="ExternalInput")
    d_out = nc.dram_tensor("out", [VC, H], dt.float32, kind="ExternalOutput")

    with tc:
        with (
            tc.tile_pool(name="const", bufs=1) as cpool,
            tc.tile_pool(name="persist", bufs=1) as ppool,
        ):
            t_w1f = cpool.tile([CHEM_IN, H], dt.bfloat16)
            nc.sync.dma_start(out=t_w1f[:], in_=d_w1f[:])
            t_b1f = cpool.tile([H, 1], dt.float32)
            nc.sync.dma_start(out=t_b1f[:], in_=d_b1f[:])
            t_w2h_f = cpool.tile([H, H], dt.bfloat16)
            nc.sync.dma_start(out=t_w2h_f[:], in_=d_w2h_f[:])
            t_w2h_c = cpool.tile([H, H], dt.bfloat16)
            nc.sync.dma_start(out=t_w2h_c[:], in_=d_w2h_c[:])
            t_biasf = cpool.tile([1, 4 * H], dt.bfloat16)
            nc.sync.dma_start(out=t_biasf[:], in_=d_biasf[:])
            t_biasc = cpool.tile([1, 4 * H], dt.bfloat16)
            nc.sync.dma_start(out=t_biasc[:], in_=d_biasc[:])
            t_ones = cpool.tile([1, P], dt.bfloat16)
            nc.sync.dma_start(out=t_ones[:], in_=d_ones[:])

            # persistent accumulation target: h_chem^T per vertex [H, VC]
            t_hcv = ppool.tile([H, VC], dt.bfloat16)

            with (
                tc.tile_pool(name="chem_in", bufs=3) as chpool,
                tc.tile_pool(name="mask_in", bufs=3) as mkpool,
                tc.tile_pool(name="h1", bufs=4) as h1pool,
                tc.tile_pool(name="tnh", bufs=3) as tpool,
                tc.tile_pool(name="g2", bufs=3) as gpool2,
                tc.tile_pool(name="psA", bufs=2, space="PSUM") as psA,
                tc.tile_pool(name="psF", bufs=2, space="PSUM") as psF,
                tc.tile_pool(name="psC", bufs=2, space="PSUM") as psC,
                tc.tile_pool(name="psS", bufs=2, space="PSUM") as psS,
            ):
                # Software pipeline: step i runs mm1(i) / mm2+gate(i-1) /
                # scatter(i-2) so the PE never waits on same-step results.
                seg_acc = {}
                ct = None
                mts = {}
                h1s = {}
                pfs = {}
                pcs = {}
                g2s = {}
                for i in range(n_st + 3):
                    st = i
                    if st < n_st:
                        if st % CH == 0:
                            ct = chpool.tile([CHEM_IN, CH * ST], dt.bfloat16,
                                             tag="ct")
                            nc.sync.dma_start(
                                out=ct[:], in_=d_chemT[:, st * ST:(st + CH) * ST])
                            mts[st // CH] = mkpool.tile(
                                [P, CH * 4 * W], dt.bfloat16, tag="mt",
                                name=f"mt_{st // CH}")
                            nc.sync.dma_start(
                                out=mts[st // CH][:],
                                in_=d_maskT[:, st * 4 * W:(st + CH) * 4 * W])
                        cs = (st % CH) * ST
                        p1 = psA.tile([P, ST], dt.float32, tag="p1")
                        nc.tensor.matmul(out=p1[:], lhsT=t_w1f[:],
                                         rhs=ct[:, cs:cs + ST],
                                         start=True, stop=True)
                        h1s[st] = h1pool.tile([P, ST], dt.bfloat16, tag="h1",
                                              name=f"h1_{st}")
                        nc.scalar.activation(h1s[st][:], p1[:], AF.Silu,
                                             bias=t_b1f[:, :1])
                    sm = i - 1    # mm2 stage (PE only)
                    if 0 <= sm < n_st:
                        h1 = h1s[sm]
                        # filter half: bias rides a K=1 ones-row matmul
                        pfs[sm] = psF.tile([P, ST], dt.float32, tag="pf",
                                           name=f"pf_{sm}")
                        pf = pfs[sm]
                        nc.tensor.matmul(out=pf[:], lhsT=t_ones[:],
                                         rhs=t_biasf[:], start=True, stop=False)
                        for k in range(4):
                            nc.tensor.matmul(out=pf[:, k * H:(k + 1) * H],
                                             lhsT=h1[:, k * P:(k + 1) * P],
                                             rhs=t_w2h_f[:], start=False,
                                             stop=(k == 3))
                        # core half
                        pcs[sm] = psC.tile([P, ST], dt.float32, tag="pc",
                                           name=f"pc_{sm}")
                        pc = pcs[sm]
                        nc.tensor.matmul(out=pc[:], lhsT=t_ones[:],
                                         rhs=t_biasc[:], start=True, stop=False)
                        for k in range(4):
                            nc.tensor.matmul(out=pc[:, k * H:(k + 1) * H],
                                             lhsT=h1[:, k * P:(k + 1) * P],
                                             rhs=t_w2h_c[:], start=False,
                                             stop=(k == 3))
                        del h1s[sm]
                    sg = i - 2    # tanh + gate stage (ACT + DVE)
                    if 0 <= sg < n_st:
                        tnh = tpool.tile([P, ST], dt.bfloat16, tag="tnh")
                        nc.scalar.activation(tnh[:], pfs[sg][:], AF.Tanh)
                        g2s[sg] = gpool2.tile([P, ST], dt.bfloat16, tag="g2",
                                              name=f"g2_{sg}")
                        nc.vector._custom_dve(GATE_SP, out=g2s[sg][:],
                                              in0=pcs[sg][:],
                                              in1=tnh[:], s0=sc0, s1=sc1,
                                              imm2=sc2)
                        del pfs[sg]
                        del pcs[sg]
                    sc = i - 3    # scatter stage
                    if 0 <= sc < n_st:
                        g2 = g2s[sc]
                        mt = mts[sc // CH]
                        for k in range(4):
                            t_idx = sc * 4 + k
                            win, first, last = tile_win[t_idx]
                            pair = win // 2
                            lo = win % 2      # half of the paired PSUM bank
                            mc = (t_idx % (CH * 4)) * W
                            if first and lo == 0:
                                seg_acc[pair] = psS.tile(
                                    [P, 2 * W], dt.float32, tag="segacc",
                                    name=f"segacc_{pair}")
                            # window 2p fills cols [0,W) with start=True on its
                            # first tile (clears the bank); window 2p+1 fills
                            # cols [W,2W) with start=False -- per-element
                            # has_written makes the first write an overwrite.
                            nc.tensor.matmul(
                                out=seg_acc[pair][:, lo * W:(lo + 1) * W],
                                lhsT=g2[:, k * P:(k + 1) * P],
                                rhs=mt[:, mc:mc + W],
                                start=(first and lo == 0),
                                stop=(last and lo == 1),
                                skip_group_check=True)
                            if last and lo == 1:
                                nc.vector.tensor_copy(
                                    out=t_hcv[:, (win - 1) * W:(win + 1) * W],
                                    in_=seg_acc[pair][:])
                                del seg_acc[pair]
                        del g2s[sc]
                        if sc % CH == CH - 1:
                            del mts[sc // CH]

            # ---------------- vertex phase (bf16) ----------------
            with (
                tc.tile_pool(name="geom_in", bufs=2) as gpool,
                tc.tile_pool(name="vtmp", bufs=3) as vtpool,
                tc.tile_pool(name="vout", bufs=3) as vopool,
                tc.tile_pool(name="psV", bufs=1, space="PSUM") as psV,
                tc.tile_pool(name="vconst", bufs=1) as vcpool,
            ):
                t_wg1f = vcpool.tile([GEOM_IN, GH], dt.bfloat16)
                nc.sync.dma_start(out=t_wg1f[:], in_=d_wg1f[:])
                t_bg1f = vcpool.tile([GH, 1], dt.float32)
                nc.sync.dma_start(out=t_bg1f[:], in_=d_bg1f[:])
                t_wg2f = vcpool.tile([GH, GH], dt.bfloat16)
                nc.sync.dma_start(out=t_wg2f[:], in_=d_wg2f[:])
                t_bg2f = vcpool.tile([GH, 1], dt.float32)
                nc.sync.dma_start(out=t_bg2f[:], in_=d_bg2f[:])
                t_wf1f_a = vcpool.tile([H, H], dt.bfloat16)
                nc.sync.dma_start(out=t_wf1f_a[:], in_=d_wf1f_a[:])
                t_wf1f_b = vcpool.tile([GH, H], dt.bfloat16)
                nc.sync.dma_start(out=t_wf1f_b[:], in_=d_wf1f_b[:])
                t_bf1f = vcpool.tile([H, 1], dt.float32)
                nc.sync.dma_start(out=t_bf1f[:], in_=d_bf1f[:])
                t_wf2f = vcpool.tile([H, H], dt.bfloat16)
                nc.sync.dma_start(out=t_wf2f[:], in_=d_wf2f[:])
                t_bf2f = vcpool.tile([H, 1], dt.float32)
                nc.sync.dma_start(out=t_bf2f[:], in_=d_bf2f[:])
                t_ident = vcpool.tile([P, P], dt.bfloat16)
                nc.sync.dma_start(out=t_ident[:], in_=d_ident[:])

                for base in range(0, VC, ST):
                    Wc = min(ST, VC - base)
                    sl = slice(base, base + Wc)
                    gt = gpool.tile([GEOM_IN, Wc], dt.bfloat16, tag="gt")
                    nc.sync.dma_start(out=gt[:], in_=d_geomT[:, sl])
                    pg1 = psV.tile([GH, Wc], dt.float32, tag="pg1")
                    nc.tensor.matmul(out=pg1[:], lhsT=t_wg1f[:], rhs=gt[:],
                                     start=True, stop=True)
                    g1s = vtpool.tile([GH, Wc], dt.bfloat16, tag="g1s")
                    nc.scalar.activation(g1s[:], pg1[:], AF.Silu, bias=t_bg1f[:, :1])
                    pg2 = psV.tile([GH, Wc], dt.float32, tag="pg2")
                    nc.tensor.matmul(out=pg2[:], lhsT=t_wg2f[:], rhs=g1s[:],
                                     start=True, stop=True)
                    hg = vtpool.tile([GH, Wc], dt.bfloat16, tag="hg")
                    nc.scalar.activation(hg[:], pg2[:], AF.Identity, bias=t_bg2f[:, :1])
                    # feat mlp
                    pf1 = psV.tile([H, Wc], dt.float32, tag="pf1", bufs=2)
                    nc.tensor.matmul(out=pf1[:], lhsT=t_wf1f_a[:],
                                     rhs=t_hcv[:, sl],
                                     start=True, stop=False)
                    nc.tensor.matmul(out=pf1[:], lhsT=t_wf1f_b[:], rhs=hg[:],
                                     start=False, stop=True)
                    x1 = vtpool.tile([H, Wc], dt.bfloat16, tag="x1")
                    nc.scalar.activation(x1[:], pf1[:], AF.Silu, bias=t_bf1f[:, :1])
                    pf2 = psV.tile([H, Wc], dt.float32, tag="pf2", bufs=2)
                    nc.tensor.matmul(out=pf2[:], lhsT=t_wf2f[:], rhs=x1[:],
                                     start=True, stop=True)
                    outT = vtpool.tile([H, Wc], dt.bfloat16, tag="outT")
                    nc.scalar.activation(outT[:], pf2[:], AF.Identity,
                                         bias=t_bf2f[:, :1])
                    for k in range(Wc // P):
                        trv = psV.tile([P, P], dt.bfloat16, tag="trv", bufs=2)
                        nc.tensor.transpose(
                            out=trv[:], in_=outT[:, k * P:(k + 1) * P],
                            identity=t_ident[:])
                        ov = vopool.tile([P, H], dt.float32, tag="ov")
                        nc.vector.tensor_copy(out=ov[:], in_=trv[:])
                        nc.sync.dma_start(
                            out=d_out[base + k * P: base + (k + 1) * P, :],
                            in_=ov[:])

    nc.compile()
    if trace_sim:
        ents = [e for e in tc._perfetto_entries if e[2] is not None]
        if ents:
            t0 = min(e[1] for e in ents)
            t1 = max(e[2] for e in ents)
            print(f"[sim] estimated makespan: {(t1 - t0) / 1000:.1f} us")
            nc._sim_makespan_ns = t1 - t0
    return nc


def kernel(chem_feats, geom_feats, nbr_vids,
           w1, b1, bn1, w2, b2, bn2,
           wg1, bg1, bng1, wg2, bg2, bng2,
           wf1, bf1, bnf1, wf2, bf2, bnf2):
    chem_feats = np.asarray(chem_feats, dtype=np.float32)
    geom_feats = np.asarray(geom_feats, dtype=np.float32)
    nbr_vids = np.asarray(nbr_vids)
    weights = tuple(np.asarray(w, dtype=np.float32) for w in (
        w1, b1, bn1, w2, b2, bn2, wg1, bg1, bng1, wg2, bg2, bng2,
        wf1, bf1, bnf1, wf2, bf2, bnf2))

    dims, T_w, consts, per_core = _host_prep(
        chem_feats, geom_feats, nbr_vids, weights)

    key = (dims["E_pad"], T_w)
    if key not in _cache:
        _cache[key] = _build_nc(dims, T_w)
    nc = _cache[key]

    base = dict(consts)
    in_maps = []
    for c in range(NCORES):
        m = dict(base)
        m["chemT"] = per_core["chemT"][c]
        m["maskT"] = per_core["maskT"][c]
        m["geomT"] = per_core["geomT"][c]
        in_maps.append(m)

    global LAST_RESULT
    if TRACE:
        res = run_bass_kernel_spmd(nc, in_maps, core_ids=list(range(NCORES)),
                                   trace=True, tmpdir="/tmp/bass_trace")
    else:
        res = run_bass_kernel_spmd(nc, in_maps, core_ids=list(range(NCORES)))
    LAST_RESULT = res
    out = np.concatenate([r["out"] for r in res.results], axis=0)
    return out.astype(np.float32)
